# revision 15
# baseline (speedup 1.0000x reference)
"""Trainium2 Bass kernel for BoundConvexViolationProjection.

Problem (hardcoded from the reference):
  x [32,8,512] f32, A [32,8,512,512] f32, b [32,8,512] f32, var_mask [32,512] f32 (ones)
  Iterate (up to MAX_ITER=100):
      r    = einsum('bsn,bsmn->bsm', x, A) - b
      viol = relu(r) - relu(-r - DELTA)
      g    = einsum('bsm,bsmn->bsn', viol, A)
      tv   = sum(relu(r), -1);  active = tv >= DELTA
      x    = max(where(active, x - LR*g/(|g|+EPS), x), 0)
  while any(active).  Key fact: per-(b,s) rows freeze once inactive (x stops
  changing => active stays false), so running the body a fixed MAX_ITER times
  with per-row gating is EXACTLY equivalent to the reference while_loop.

Sharding: data-parallel over batch B across 8 cores (4 batches = 32 (b,s)
pairs per core); the loop state is fully local, no collectives.

Per-core kernel strategy (memory-regime):
  Everything lives in the TRANSPOSED domain: state xT[n, pair], residual
  rT[m, pair], grad gT[n, pair] as dense PSUM/SBUF columns.  Both einsums
  are weight-stationary matmuls: the 128x128 A-block is the stationary
  operand, the x/viol column [128,1] is the moving operand.  With FD=1 the
  kernel is LDWEIGHTS-bound, so A is stored as fp8 E3M4 (float8e3): FWL
  loads fp8 weights at 4 elem/cycle (2x bf16), and BOTH layouts (A^T
  n-major for the residual, A m-major for the grad) sit fully resident in
  SBUF (8+8 MiB of 24) -- zero HBM traffic inside the loop.  Moving
  operands stay bf16.

  Engine queues are strictly in-order, so the software pipeline is laid
  out so that EVERY PE instruction depends only on DVE/ACT work emitted
  in the PREVIOUS step (chunk-stage), never the current one: PE per step
  runs RES(c)[128 MM] | MERGED-REDUCE(c-2)[1 MM] | OUTER(c-3)[1 MM] |
  GRAD(c-1)[128 MM] back-to-back, while DVE retires scale(c-2),
  update(c-3), gsq(c-1), glue1(c) under them.  tv and |g|^2 column sums
  share one ones-vector matmul ([128, 2W] moving); the per-pair step
  scale is broadcast across partitions with a rank-1 outer-product
  matmul (fp8 ones stationary, bf16 coef moving).  viol uses a DVE-only
  clamp identity (viol = r - clamp(r, -DELTA, 0)), keeping ACT off the
  critical path (ACT only does the rsqrt).
fp8 E3M4 A (4 mantissa bits, max 15.5 >> max|A|=5.42) with fp32
accumulation was validated against the f32 reference in numpy: absmax
relative error ~1.3e-3 over the full 100 iterations (gate is 2e-2).
"""

import numpy as np
import ml_dtypes

import concourse.bacc as bacc
import concourse.bass as bass
import concourse.mybir as mybir
import concourse.tile as tile
from concourse.bass_utils import run_bass_kernel_spmd

BF16 = ml_dtypes.bfloat16
F8E3 = ml_dtypes.float8_e3m4

N_CORES = 8
B, S, M, N = 32, 8, 512, 512
B_LOC = B // N_CORES            # 4 batches per core
P = B_LOC * S                   # 32 (b,s) pairs per core
NT = N // 128                   # 4 n-tiles
MT = M // 128                   # 4 m-tiles
LR, DELTA = 0.005, 0.1
N_ITERS = 100
CPP = 8                         # pairs per pipeline chunk
NCH = P // CPP                  # 4 chunks
W = CPP * 4                     # 32 columns per chunk ((mt|nt, jj))


def _build_nc(n_iters=N_ITERS):
    f32 = mybir.dt.float32
    bf16 = mybir.dt.bfloat16
    f8e3 = mybir.dt.float8e3
    Sqrt = mybir.ActivationFunctionType.Sqrt
    Copy = mybir.ActivationFunctionType.Copy
    Square = mybir.ActivationFunctionType.Square
    Alu = mybir.AluOpType

    nc = bacc.Bacc("TRN2", target_bir_lowering=False)
    at_d = nc.dram_tensor("at", [P, 128, NT, 512], f8e3, kind="ExternalInput")
    ar_d = nc.dram_tensor("arows", [P, 128, MT, 512], f8e3, kind="ExternalInput")
    bt_d = nc.dram_tensor("bt", [128, NCH * W], f32, kind="ExternalInput")
    xt_d = nc.dram_tensor("x0t", [128, NCH * W], f32, kind="ExternalInput")
    id_d = nc.dram_tensor("ident", [128, 128], f32, kind="ExternalInput")
    out_d = nc.dram_tensor("xout", [P, 512], f32, kind="ExternalOutput")

    ones128 = nc.const_aps.tensor(1.0, (128, 1))  # [128,1] f32 ones (preamble)

    with tile.TileContext(nc) as tc:
        with (
            tc.tile_pool(name="resident", bufs=1) as res_pool,
            tc.tile_pool(name="glue", bufs=7) as glue_pool,
            tc.tile_pool(name="violp", bufs=3) as viol_pool,
            tc.tile_pool(name="redup", bufs=4) as redu_pool,
            tc.tile_pool(name="gpool", bufs=7) as g_pool,
            tc.tile_pool(name="xstate", bufs=2 * NCH + 2) as x_pool,
            tc.tile_pool(name="xtb", bufs=2 * NCH + 2) as xtb_pool,
            tc.tile_pool(name="rows", bufs=12) as row_pool,
            tc.tile_pool(name="mmps", bufs=5, space=bass.MemorySpace.PSUM) as mm_psum,
            tc.tile_pool(name="rowps", bufs=2, space=bass.MemorySpace.PSUM) as row_psum,
            tc.tile_pool(name="finps", bufs=1, space=bass.MemorySpace.PSUM) as fin_psum,
        ):
            # ---- persistent tiles + initial loads ----
            ar_sb = res_pool.tile([128, P, MT, 512], f8e3, tag="ar_sb")
            at_sb = res_pool.tile([128, P, NT, 512], f8e3, tag="at_sb")
            bt_sb = res_pool.tile([128, NCH * W], f32, tag="bt_sb")
            id_sb = res_pool.tile([128, 128], f32, tag="id_sb")
            cst = res_pool.tile([128, 2], f32, tag="cst")
            ones1 = res_pool.tile([1, 128], f8e3, tag="ones1")
            nc.vector.memset(cst[:, 0:1], -DELTA)
            nc.vector.memset(cst[:, 1:2], 1e-12)
            nc.vector.memset(ones1[:], 1.0)

            # init loads via SWDGE (gpsimd): one shared semaphore, so any
            # compute op depending on them needs just one wait (walrus
            # allows a single sync-wait per compute instruction).  Emitted
            # in CONSUMPTION order (x/b first, then per-chunk at|ar) so
            # iteration-0 compute starts as soon as chunk 0 arrives instead
            # of waiting out the whole ~60us init epoch.
            x_cur = [None] * NCH    # f32 [128, W] transposed state per chunk
            xb_cur = [None] * NCH   # bf16 copy for matmul rhs

            for c in range(NCH):
                xc = x_pool.tile([128, W], f32, tag="x")
                nc.gpsimd.dma_start(out=xc[:], in_=xt_d[:, c * W:(c + 1) * W])
                x_cur[c] = xc
            nc.gpsimd.dma_start(out=bt_sb[:], in_=bt_d[:])
            nc.gpsimd.dma_start(out=id_sb[:], in_=id_d[:])
            for c in range(NCH):
                xb = xtb_pool.tile([128, W], bf16, tag="xb")
                nc.vector.tensor_copy(xb[:], x_cur[c][:])
                xb_cur[c] = xb

            # PE warm-up: one trash matmul depending on the x loads only --
            # folds the early init epoch into PE's vector clock without
            # serializing iteration 0 behind the full A load.
            warm = fin_psum.tile([1, 1], f32, tag="fin")
            nc.tensor.matmul(warm[:], x_cur[NCH - 1][:, 0:1],
                             x_cur[NCH - 1][:, 0:1], start=True, stop=True)

            for c in range(NCH):
                for j in range(c * CPP, (c + 1) * CPP):
                    nc.gpsimd.dma_start(out=at_sb[:, j], in_=at_d[j])
                for j in range(c * CPP, (c + 1) * CPP):
                    nc.gpsimd.dma_start(out=ar_sb[:, j], in_=ar_d[j])

            pr_ps = [None] * NCH

            def emit_res(c):
                prg = mm_psum.tile([128, W], f32, tag="mm")
                xb = xb_cur[c]
                for jj in range(CPP):
                    j = c * CPP + jj
                    for mt in range(MT):
                        col = mt * CPP + jj
                        for nt in range(NT):
                            nc.tensor.matmul(
                                prg[:, col:col + 1],
                                at_sb[:, j, nt, mt * 128:(mt + 1) * 128],
                                xb[:, nt * CPP + jj: nt * CPP + jj + 1],
                                start=(nt == 0),
                                stop=(nt == NT - 1),
                            )
                pr_ps[c] = prg

            def emit_glue1(c):
                # DVE-only: r = prg - b; rp = relu(r) into redu[:, :W];
                # violT = r - clamp(r, -DELTA, 0)  (== relu(r) - relu(-r-D))
                prg = pr_ps[c]
                redu = redu_pool.tile([128, 2 * W], f32, tag="redu")
                r_sb = glue_pool.tile([128, W], f32, tag="glue")
                nc.vector.tensor_tensor(
                    r_sb[:], prg[:], bt_sb[:, c * W:(c + 1) * W], Alu.subtract)
                nc.vector.tensor_scalar(out=redu[:, 0:W], in0=r_sb[:],
                                        scalar1=0.0, scalar2=None, op0=Alu.max)
                rc = glue_pool.tile([128, W], f32, tag="glue")
                nc.vector.tensor_scalar(out=rc[:], in0=r_sb[:], scalar1=0.0,
                                        scalar2=-DELTA, op0=Alu.min, op1=Alu.max)
                violT = viol_pool.tile([128, W], bf16, tag="viol")
                nc.vector.tensor_tensor(violT[:], r_sb[:], rc[:], Alu.subtract)
                return violT, redu

            def emit_grad(c, violT):
                pgg = mm_psum.tile([128, W], f32, tag="mm")
                for jj in range(CPP):
                    j = c * CPP + jj
                    for nt in range(NT):
                        col = nt * CPP + jj
                        for mt in range(MT):
                            nc.tensor.matmul(
                                pgg[:, col:col + 1],
                                ar_sb[:, j, mt, nt * 128:(nt + 1) * 128],
                                violT[:, mt * CPP + jj: mt * CPP + jj + 1],
                                start=(mt == 0),
                                stop=(mt == MT - 1),
                            )
                return pgg

            def emit_gsq(c, pgg, redu):
                # gT copy for the update; |g|^2 terms into redu[:, W:].
                # Both on ACT: they are the only ops that wait on GRAD-end,
                # and putting them on DVE head-of-line-blocks the DVE queue
                # (scale/update/glue) behind a ~4us semaphore wait.  Square
                # first: MERGED (PE) waits only on it, not on the gT copy.
                gT = g_pool.tile([128, W], f32, tag="gt")
                nc.scalar.activation(redu[:, W:2 * W], pgg[:], Square)
                nc.scalar.activation(gT[:], pgg[:], Copy)
                return gT

            def emit_merged_mm(redu):
                # one ones-vector matmul: cols 0..W-1 -> tv partials,
                # cols W..2W-1 -> |g|^2 partials
                ts_ps = row_psum.tile([1, 2 * W], f32, tag="rowps")
                nc.tensor.matmul(ts_ps[:], ones128, redu[:],
                                 start=True, stop=True)
                return ts_ps

            def emit_scale(ts_ps):
                # [1,2W] -> [1,2*CPP]: sum the 4 tile-partials per pair
                red = row_pool.tile([1, 2 * CPP], f32, tag="row")
                nc.vector.tensor_reduce(
                    red[:].rearrange("p (g j) -> p g j", g=2),
                    ts_ps[:].rearrange("p (g m j) -> p g j m", g=2, j=CPP),
                    axis=mybir.AxisListType.X, op=Alu.add)
                mlr = row_pool.tile([1, CPP], f32, tag="row")
                nc.vector.tensor_scalar(out=mlr[:], in0=red[:, 0:CPP],
                                        scalar1=DELTA, scalar2=LR,
                                        op0=Alu.is_ge, op1=Alu.mult)
                # sqrt(s2 + 1e-12): guards g==0 (reference adds EPS=1e-6 to
                # |g|; the difference is far below bf16 noise)
                s = row_pool.tile([1, CPP], f32, tag="row")
                nc.scalar.activation(s[:], red[:, CPP:2 * CPP], Sqrt,
                                     bias=cst[:1, 1:2])
                inv = row_pool.tile([1, CPP], f32, tag="row")
                nc.vector.reciprocal(inv[:], s[:])
                coef = row_pool.tile([1, CPP], f32, tag="row")
                nc.vector.tensor_tensor(coef[:], mlr[:], inv[:], Alu.mult)
                coef4 = row_pool.tile([1, W], bf16, tag="row4")
                for nt in range(NT):
                    nc.vector.tensor_copy(coef4[:, nt * CPP:(nt + 1) * CPP],
                                          coef[:])
                return coef4

            def emit_outer(coef4):
                cb_ps = mm_psum.tile([128, W], f32, tag="mm")
                nc.tensor.matmul(cb_ps[:], ones1[:], coef4[:],
                                 start=True, stop=True)
                return cb_ps

            def emit_update(c, gT, cb_ps):
                upd = glue_pool.tile([128, W], f32, tag="glue")
                nc.vector.tensor_tensor(upd[:], gT[:], cb_ps[:], Alu.mult)
                xn = glue_pool.tile([128, W], f32, tag="glue")
                nc.vector.tensor_tensor(xn[:], x_cur[c][:], upd[:], Alu.subtract)
                xnew = x_pool.tile([128, W], f32, tag="x")
                nc.vector.tensor_scalar(out=xnew[:], in0=xn[:], scalar1=0.0,
                                        scalar2=None, op0=Alu.max)
                xb = xtb_pool.tile([128, W], bf16, tag="xb")
                nc.vector.tensor_copy(xb[:], xnew[:])
                x_cur[c] = xnew
                xb_cur[c] = xb

            # ---- main loop: software-pipelined chunk emission ----
            # Per-chunk schedule (steps): RES+glue1 @s | GRAD+gsq @s+1 |
            # MERGED+scale @s+2 | OUTER+update @s+3 | next RES @s+4.
            # Each engine's in-order queue per step only waits on the OTHER
            # engine's previous-step output, so PE never stalls on DVE.
            pend_g = None   # (c, violT, redu)   from glue1@s
            pend_m = None   # (c, gT, redu)      from gsq@s
            pend_u = None   # (c, gT, coef4)     from scale@s
            steps = n_iters * NCH
            for step in range(steps + 3):
                cur = step % NCH if step < steps else None
                # ---- PE queue ----
                if cur is not None:
                    emit_res(cur)                     # PE 128 MM
                if pend_m is not None:
                    mc, gT_m, redu_m = pend_m
                    ts_ps = emit_merged_mm(redu_m)    # PE 1 MM
                if pend_u is not None:
                    uc, gT_u, coef4_u = pend_u
                    cb_ps = emit_outer(coef4_u)       # PE 1 MM
                if pend_g is not None:
                    gc, violT_g, redu_g = pend_g
                    pgg = emit_grad(gc, violT_g)      # PE 128 MM
                # ---- DVE/ACT queue ----
                # The scheduler's sim over-estimates DVE latency and will
                # otherwise slide the state-critical chains (violT, xb) a
                # step late, stalling GRAD/RES on HW.  high_priority pins
                # them at the front of the ready heap; the scale chain has
                # a full step of slack and stays at normal priority.
                if pend_m is not None:
                    coef4 = emit_scale(ts_ps)         # dep MERGED@s
                    new_pend_u = (mc, gT_m, coef4)
                else:
                    new_pend_u = None
                with tc.high_priority(offset=1120):
                    if pend_u is not None:
                        emit_update(uc, gT_u, cb_ps)  # dep OUTER@s
                    if pend_g is not None:
                        gT = emit_gsq(gc, pgg, redu_g)  # dep GRAD@s
                        new_pend_m = (gc, gT, redu_g)
                    else:
                        new_pend_m = None
                    if cur is not None:
                        violT, redu = emit_glue1(cur)   # dep RES@s
                        pend_g = (cur, violT, redu)
                    else:
                        pend_g = None
                pend_m = new_pend_m
                pend_u = new_pend_u

            # ---- store result: un-transpose once ----
            for c in range(NCH):
                pT = fin_psum.tile([W, 128], f32, tag="fin")
                nc.tensor.transpose(pT[:], x_cur[c][:], id_sb[:])
                fin = glue_pool.tile([W, 128], f32, tag="fin_sb")
                nc.vector.tensor_copy(fin[:], pT[:])
                for nt in range(NT):
                    nc.sync.dma_start(
                        out=out_d[c * CPP:(c + 1) * CPP,
                                  nt * 128:(nt + 1) * 128],
                        in_=fin[nt * CPP:(nt + 1) * CPP, :],
                    )

    nc.compile()
    return nc


_NC_CACHE = {}


def _get_nc(n_iters=N_ITERS):
    if n_iters not in _NC_CACHE:
        _NC_CACHE[n_iters] = _build_nc(n_iters)
    return _NC_CACHE[n_iters]


def _tcols(v):
    """[P, 512] -> [128, NCH*W] with col = c*W + t*CPP + jj, t = 128-block."""
    return np.ascontiguousarray(
        v.reshape(NCH, CPP, 4, 128).transpose(3, 0, 2, 1).reshape(128, NCH * W))


def _prep_core_inputs(Ac, bc, xc):
    """Ac [P,512,512] f32, bc [P,512], xc [P,512] -> per-core input map."""
    # at[j, p, nt, m] = Ac[j, m, nt*128+p]
    at = np.ascontiguousarray(
        Ac.reshape(P, M, NT, 128).transpose(0, 3, 2, 1)
    ).astype(F8E3)
    # arows[j, p, mt, n] = Ac[j, mt*128+p, n]
    ar = np.ascontiguousarray(
        Ac.reshape(P, MT, 128, N).transpose(0, 2, 1, 3)
    ).astype(F8E3)
    return {
        "at": at,
        "arows": ar,
        "bt": _tcols(np.asarray(bc, dtype=np.float32)),
        "x0t": _tcols(np.asarray(xc, dtype=np.float32)),
        "ident": np.eye(128, dtype=np.float32),
    }


def kernel(x, A, b, var_mask):
    x = np.asarray(x, dtype=np.float32)
    A = np.asarray(A, dtype=np.float32)
    b = np.asarray(b, dtype=np.float32)
    var_mask = np.asarray(var_mask, dtype=np.float32)

    nc = _get_nc()
    in_maps = []
    for c in range(N_CORES):
        bs = slice(c * B_LOC, (c + 1) * B_LOC)
        in_maps.append(
            _prep_core_inputs(
                A[bs].reshape(P, M, N), b[bs].reshape(P, M), x[bs].reshape(P, N)
            )
        )

    res = run_bass_kernel_spmd(nc, in_maps, list(range(N_CORES)))

    out = np.empty((B, S, N), dtype=np.float32)
    for c in range(N_CORES):
        out[c * B_LOC:(c + 1) * B_LOC] = res.results[c]["xout"].reshape(B_LOC, S, N)
    # reference returns x_fin * var_mask (var_mask is ones per the input spec;
    # this also keeps the general contract for any mask values)
    out *= var_mask[:, None, :]
    return out
